# revision 11
# baseline (speedup 1.0000x reference)
"""Multi-head attention (QKV proj + RoPE + softmax attention + o-proj) on 8
Trainium2 NeuronCores.

Sharding: data-parallel over batch (B=2) x tensor-parallel over heads
(16 heads -> 4 groups of 4). Core c handles batch c//4, heads 4*(c%4)..+4.
qkv_proj is column-parallel, o_proj row-parallel; each core returns a
partial o-proj output and the host sums the 4 partials per batch.

All matmuls run in fp16 (full PE speed, ~8x better precision than bf16;
all values here are well inside fp16 range) with fp32 PSUM accumulation.

Schedule notes (per core):
 - x is fed transposed: xT [HID, N]; weight/x loads are chunked so compute
   starts as soon as the first chunks land, with a short dummy-matmul
   warm-up keeping the PE clock (HAM) at full rate through the DMA lead-in.
 - q,k are produced as [dh, tok]; scores are computed transposed
   (S^T = k^T q) so softmax's k-reduction lands on matmul-friendly axes.
 - softmax: exp on the scalar engine (scale folded in); the denominator is
   a pairwise fp16 tree-sum of the exp tiles on DVE plus one all-ones
   matmul partition-reduce; normalize via fast approximate reciprocal.
 - The PE is kept at full duty through phase B by interleaving one extra
   matmul per kt slot: o-proj matmuls of the previous token stripe (and,
   for the first stripe, the deferred stripe-3 q-projection chains).
"""

import sys

if "/opt/trn_rl_repo" not in sys.path:
    sys.path.insert(0, "/opt/trn_rl_repo")

import numpy as np

import concourse.bass as bass
import concourse.mybir as mybir
import concourse.tile as tile
from concourse.tile import add_dep_helper
from concourse import bacc

B, N, HID, H = 2, 2048, 2048, 16
DH = 128
HPC = 4  # heads per core
P = 128
F16 = mybir.dt.float16
F32 = mybir.dt.float32
SCALE = 1.0 / float(np.sqrt(DH))

_NC_CACHE = [None]


def build_nc():
    nc = bacc.Bacc(None, target_bir_lowering=False)

    xT = nc.dram_tensor("xT", [HID, N], F16, kind="ExternalInput")
    wqkT = nc.dram_tensor("wqkT", [HID, 2 * HPC * DH], F16, kind="ExternalInput")
    wvT = nc.dram_tensor("wvT", [HID, HPC * DH], F16, kind="ExternalInput")
    woT = nc.dram_tensor("woT", [HPC * DH, HID], F16, kind="ExternalInput")
    cosT = nc.dram_tensor("cosT", [DH, N], F32, kind="ExternalInput")
    sinT = nc.dram_tensor("sinT", [DH, N], F32, kind="ExternalInput")
    outT = nc.dram_tensor("outT", [HID, N], F32, kind="ExternalOutput")

    KT = HID // P  # 16 contraction tiles over hid
    NT = N // P  # 16 token tiles
    NS = N // 512  # 4 token stripes
    MQK = 2 * HPC  # 8 output dim-tiles for q+k

    mult = mybir.AluOpType.mult
    add = mybir.AluOpType.add
    Exp = mybir.ActivationFunctionType.Exp

    xT3 = xT[:].rearrange("(kt p) n -> kt p n", p=P)
    wvT3 = wvT[:].rearrange("(kt p) m -> kt p m", p=P)

    with tile.TileContext(nc) as tc:
        with (
            tc.tile_pool(name="const", bufs=1) as const,
            tc.tile_pool(name="persist", bufs=1) as persist,
        ):
            ones_sb = const.tile([P, P], F16)
            nc.vector.memset(ones_sb[:], 1.0)
            warm_src = const.tile([P, 512], F16, tag="warmsrc")
            nc.vector.memset(warm_src[:], 0.0)

            # -------- startup loads, chunked --------
            x0_tiles = []
            for kt in range(KT):
                xk = persist.tile([P, 512], F16, tag=f"x0_{kt}", name=f"x0_{kt}")
                nc.sync.dma_start(xk[:], xT3[kt, :, 0:512])
                x0_tiles.append(xk)
            # wqk split per 128-wide output block: the m=0 chain can start
            # after just 0.5MB instead of the whole 4MB.
            wqkm_tiles = []

            def load_wqkm(m):
                wk = const.tile(
                    [P, KT, P], F16, tag=f"wqkm{m}", name=f"wqkm{m}"
                )
                nc.sync.dma_start(
                    wk[:],
                    wqkT[:, m * P : (m + 1) * P].rearrange(
                        "(kt p) c -> p kt c", p=P
                    ),
                )
                wqkm_tiles.append(wk)

            cos_tiles, sin_tiles = [], []

            def load_tables(t):
                ck = const.tile([P, 512], F32, tag=f"cos{t}", name=f"cos{t}")
                nc.sync.dma_start(ck[:], cosT[:, t * 512 : (t + 1) * 512])
                cos_tiles.append(ck)
                sk = const.tile([P, 512], F32, tag=f"sin{t}", name=f"sin{t}")
                nc.sync.dma_start(sk[:], sinT[:, t * 512 : (t + 1) * 512])
                sin_tiles.append(sk)

            load_wqkm(0)
            load_wqkm(1)
            load_tables(0)  # RoPE of the first chain needs stripe-0 tables
            for m in range(2, MQK):
                load_wqkm(m)

            # persistent intermediates
            qk_tiles = [
                persist.tile([P, N], F16, tag=f"qk{m}", name=f"qk{m}")
                for m in range(MQK)
            ]
            v_sb = persist.tile([P, NT, HPC * DH], F16, tag="v")

            # ---------------- Phase A: QKV + RoPE ----------------
            # (stripe 3's q chains are deferred into phase B's filler slots)
            with (
                tc.tile_pool(name="aphase", bufs=2) as aphase,
                tc.tile_pool(name="psumA", bufs=3, space="PSUM") as psumA,
            ):
                wv_tiles = []
                for kt in range(KT):
                    wk = aphase.tile(
                        [P, HPC * DH], F16, tag=f"wv{kt}", name=f"wv{kt}", bufs=1
                    )
                    nc.sync.dma_start(wk[:], wvT3[kt])
                    wv_tiles.append(wk)
                for t in range(1, NS):
                    load_tables(t)

                warm_ps = psumA.tile([P, 512], F32, tag="warm", bufs=1)
                for _ in range(48):
                    nc.tensor.matmul(
                        warm_ps[:], ones_sb[:], warm_src[:], start=True, stop=True
                    )

                for s in range(NS):
                    if s == 0:
                        x_tiles = x0_tiles
                    else:
                        xs = aphase.tile(
                            [P, KT, 512], F16, tag="xs", name="xs", bufs=2
                        )
                        nc.sync.dma_start(
                            xs[:],
                            xT[:, s * 512 : (s + 1) * 512].rearrange(
                                "(kt p) n -> p kt n", p=P
                            ),
                        )
                        x_tiles = [xs[:, kt, :] for kt in range(KT)]
                    sl = slice(s * 512, (s + 1) * 512)
                    m_list = range(MQK) if s < NS - 1 else range(HPC, MQK)
                    for m in m_list:
                        ps = psumA.tile([P, 512], F32, tag="ps")
                        for kt in range(KT):
                            nc.tensor.matmul(
                                ps[:],
                                wqkm_tiles[m][:, kt, :],
                                x_tiles[kt][:],
                                start=(kt == 0),
                                stop=(kt == KT - 1),
                            )
                        # RoPE: out = ps*cos + rot(ps)*sin_signed
                        rot = aphase.tile([P, 512], F32, tag="rot")
                        nc.scalar.copy(rot[0:64, :], ps[64:128, :])
                        nc.scalar.copy(rot[64:128, :], ps[0:64, :])
                        t1 = aphase.tile([P, 512], F32, tag="t1")
                        nc.vector.tensor_tensor(t1[:], rot[:], sin_tiles[s][:], mult)
                        t2 = aphase.tile([P, 512], F32, tag="t2")
                        nc.vector.tensor_tensor(t2[:], ps[:], cos_tiles[s][:], mult)
                        nc.vector.tensor_tensor(
                            qk_tiles[m][:, sl], t1[:], t2[:], add
                        )
                    for tt in range(4):
                        tok = s * 4 + tt
                        psv = psumA.tile([P, 512], F32, tag="psv", bufs=2)
                        for kt in range(KT):
                            nc.tensor.matmul(
                                psv[:],
                                x_tiles[kt][:, tt * P : (tt + 1) * P],
                                wv_tiles[kt][:],
                                start=(kt == 0),
                                stop=(kt == KT - 1),
                            )
                        nc.scalar.copy(v_sb[:, tok, :], psv[:])

            # ---------------- Phases B (attention) + C (o-proj) ----------
            with tc.tile_pool(name="late", bufs=3) as late:
                # x stripe-3 reload (chunked) for the deferred q chains
                x3r_tiles = []
                for kt in range(KT):
                    xk = late.tile(
                        [P, 512], F16, tag=f"x3r{kt}", name=f"x3r{kt}", bufs=1
                    )
                    nc.sync.dma_start(
                        xk[:], xT3[kt, :, (NS - 1) * 512 : NS * 512]
                    )
                    x3r_tiles.append(xk)
                wo_sb = late.tile([P, HPC, HID], F16, tag="wo", bufs=1)
                nc.sync.dma_start(
                    wo_sb[:], woT[:].rearrange("(ht p) o -> p ht o", p=P)
                )
                attn_tiles = [
                    late.tile([P, N], F16, tag=f"attn{h}", name=f"attn{h}", bufs=1)
                    for h in range(HPC)
                ]

                with tc.tile_pool(name="psumB", bufs=2, space="PSUM") as psumB:
                    pending = []  # deferred denominator/normalize closure args

                    def emit_denorm(accs, outp, h, qsl):
                        stride = 1
                        while stride < 8:
                            for i in range(0, 8, 2 * stride):
                                nc.vector.tensor_tensor(
                                    accs[i][:], accs[i][:], accs[i + stride][:], add
                                )
                            stride *= 2
                        den = psumB.tile([P, 512], F32, tag="den", bufs=1, name="den")
                        nc.tensor.matmul(
                            den[:], ones_sb[:], accs[0][:], start=True, stop=True
                        )
                        rec = late.tile([P, 512], F32, tag="rec", bufs=2)
                        nc.vector.reciprocal_approx_fast(rec[:], den[:])
                        nc.vector.tensor_tensor(
                            attn_tiles[h][:, qsl], outp[:], rec[:], mult
                        )

                    def make_c_fillers(ts, tag="cps"):
                        # one closure per o-proj matmul for token stripe ts,
                        # spread through the next stripe's kt slots to keep
                        # the PE at full duty.
                        tsl = slice(ts * 512, (ts + 1) * 512)
                        state = {}
                        fillers = []

                        def mk(ho, hi, tag="cps"):
                            def emit():
                                if hi == 0:
                                    state[ho] = psumB.tile(
                                        [P, 512], F32, tag=tag, bufs=2, name="cps"
                                    )
                                nc.tensor.matmul(
                                    state[ho][:],
                                    wo_sb[:, hi, ho * P : (ho + 1) * P],
                                    attn_tiles[hi][:, tsl],
                                    start=(hi == 0),
                                    stop=(hi == HPC - 1),
                                )
                                if hi == HPC - 1:
                                    ob = late.tile([P, 512], F32, tag="co")
                                    nc.vector.tensor_copy(ob[:], state[ho][:])
                                    nc.sync.dma_start(
                                        outT[ho * P : (ho + 1) * P, tsl], ob[:]
                                    )
                            return emit

                        for ho in range(HID // P):
                            for hi in range(HPC):
                                fillers.append(mk(ho, hi, tag=tag))
                        return fillers

                    def make_q3_fillers():
                        # deferred stripe-3 q-projection chains (m=0..3),
                        # one matmul per filler slot + RoPE at chain end.
                        s3 = NS - 1
                        sl3 = slice(s3 * 512, (s3 + 1) * 512)
                        state = {}
                        fillers = []

                        def mk(m, kt):
                            def emit():
                                if kt == 0:
                                    state[m] = psumB.tile(
                                        [P, 512], F32, tag="cps", bufs=2, name="q3ps"
                                    )
                                nc.tensor.matmul(
                                    state[m][:],
                                    wqkm_tiles[m][:, kt, :],
                                    x3r_tiles[kt][:],
                                    start=(kt == 0),
                                    stop=(kt == KT - 1),
                                )
                                if kt == KT - 1:
                                    ps = state[m]
                                    rot = late.tile([P, 512], F32, tag="rot3")
                                    nc.vector.tensor_copy(rot[0:64, :], ps[64:128, :])
                                    nc.vector.tensor_copy(rot[64:128, :], ps[0:64, :])
                                    t1 = late.tile([P, 512], F32, tag="t13")
                                    nc.vector.tensor_tensor(
                                        t1[:], rot[:], sin_tiles[s3][:], mult
                                    )
                                    t2 = late.tile([P, 512], F32, tag="t23")
                                    nc.vector.tensor_tensor(
                                        t2[:], ps[:], cos_tiles[s3][:], mult
                                    )
                                    nc.vector.tensor_tensor(
                                        qk_tiles[m][:, sl3], t1[:], t2[:], add
                                    )
                            return emit

                        for m in range(HPC):
                            for kt in range(KT):
                                fillers.append(mk(m, kt))
                        return fillers

                    for qs in range(NS):
                        qsl = slice(qs * 512, (qs + 1) * 512)
                        fillers = (
                            make_c_fillers(qs - 1) if qs > 0 else make_q3_fillers()
                        )
                        fi = 0
                        for h in range(HPC):
                            kT_tile = qk_tiles[HPC + h]
                            qT_tile = qk_tiles[h]
                            outp = psumB.tile([P, 512], F32, tag="po", bufs=2)
                            accs = []
                            prev_pt = None
                            prev_pv = None
                            for kt in range(NT):
                                st = psumB.tile([P, 512], F32, tag="st", bufs=3)
                                nc.tensor.matmul(
                                    st[:],
                                    kT_tile[:, kt * P : (kt + 1) * P],
                                    qT_tile[:, qsl],
                                    start=True,
                                    stop=True,
                                )
                                pt = late.tile([P, 512], F16, tag="pt", bufs=6)
                                nc.scalar.activation(pt[:], st[:], Exp, scale=SCALE)
                                pv = nc.tensor.matmul(
                                    outp[:],
                                    v_sb[:, kt, h * DH : (h + 1) * DH],
                                    pt[:],
                                    start=(kt == 0),
                                    stop=(kt == NT - 1),
                                )
                                if kt % 2 == 0:
                                    prev_pt = pt
                                    prev_pv = pv
                                else:
                                    acc = late.tile(
                                        [P, 512], F16, tag="acc", bufs=10, name="acc"
                                    )
                                    ai = nc.vector.tensor_tensor(
                                        acc[:], prev_pt[:], pt[:], add
                                    )
                                    add_dep_helper(ai.ins, prev_pv.ins,
                                                   reason="pt SBUF contention")
                                    add_dep_helper(ai.ins, pv.ins,
                                                   reason="pt SBUF contention")
                                    accs.append(acc)
                                if kt == 3 and pending:
                                    emit_denorm(*pending.pop())
                                if (h > 0 or kt >= 6) and fi < len(fillers):
                                    fillers[fi]()
                                    fi += 1
                            pending.append((accs, outp, h, qsl))
                        while fi < len(fillers):
                            fillers[fi]()
                            fi += 1
                    emit_denorm(*pending.pop())
                    # final stripe's o-proj: alternate over the now-free psum
                    # tags for deeper pipelining of the tail burst
                    tail_f = []
                    for tag in ("cps", "po"):
                        tail_f.append(make_c_fillers(NS - 1, tag=tag))
                    for g in range(HID // P):
                        for hi in range(HPC):
                            tail_f[g % 2][g * HPC + hi]()

    nc.finalize()
    return nc


def get_nc():
    if _NC_CACHE[0] is None:
        _NC_CACHE[0] = build_nc()
    return _NC_CACHE[0]


def make_in_maps(hidden_states, cos, sin, w_qkv, w_o):
    """Build the 8 per-core input maps (host-side shard + transpose + cast)."""
    hidden_states = np.asarray(hidden_states, dtype=np.float32)
    cos = np.asarray(cos, dtype=np.float32)
    sin = np.asarray(sin, dtype=np.float32)
    w_qkv = np.asarray(w_qkv, dtype=np.float32)
    w_o = np.asarray(w_o, dtype=np.float32)

    cosT = np.ascontiguousarray(cos.T)  # [DH, N]
    sinT_signed = np.ascontiguousarray(
        np.concatenate([-sin.T[: DH // 2], sin.T[DH // 2 :]], axis=0)
    )

    xT = [
        np.ascontiguousarray(hidden_states[b].T).astype(np.float16)
        for b in range(B)
    ]

    in_maps = []
    for c in range(8):
        b, g = divmod(c, 4)
        qrows = slice(g * HPC * DH, (g + 1) * HPC * DH)
        krows = slice(HID + g * HPC * DH, HID + (g + 1) * HPC * DH)
        vrows = slice(2 * HID + g * HPC * DH, 2 * HID + (g + 1) * HPC * DH)
        wqkT = (
            np.concatenate([w_qkv[qrows], w_qkv[krows]], axis=0)
            .T.astype(np.float16)
        )
        wvT = w_qkv[vrows].T.astype(np.float16)
        woT = w_o[:, g * HPC * DH : (g + 1) * HPC * DH].T.astype(np.float16)
        in_maps.append(
            {
                "xT": xT[b],
                "wqkT": np.ascontiguousarray(wqkT),
                "wvT": np.ascontiguousarray(wvT),
                "woT": np.ascontiguousarray(woT),
                "cosT": cosT,
                "sinT": sinT_signed,
            }
        )
    return in_maps


def assemble_output(results):
    """Sum the 4 o-proj partials per batch and transpose back."""
    out = np.zeros((B, N, HID), dtype=np.float32)
    for c, res in enumerate(results):
        b = c // 4
        out[b] += res["outT"].T
    return out


def kernel(hidden_states, cos, sin, w_qkv, w_o):
    from concourse.bass_utils import run_bass_kernel_spmd

    nc = get_nc()
    in_maps = make_in_maps(hidden_states, cos, sin, w_qkv, w_o)
    res = run_bass_kernel_spmd(nc, in_maps, core_ids=list(range(8)))
    return assemble_output(res.results)


# revision 12
# speedup vs baseline: 1.0097x; 1.0097x over previous
"""Multi-head attention (QKV proj + RoPE + softmax attention + o-proj) on 8
Trainium2 NeuronCores.

Sharding: data-parallel over batch (B=2) x tensor-parallel over heads
(16 heads -> 4 groups of 4). Core c handles batch c//4, heads 4*(c%4)..+4.
qkv_proj is column-parallel, o_proj row-parallel; each core returns a
partial o-proj output and the host sums the 4 partials per batch.

All matmuls run in fp16 (full PE speed, ~8x better precision than bf16;
all values here are well inside fp16 range) with fp32 PSUM accumulation.

Schedule notes (per core):
 - x is fed transposed: xT [HID, N]; weight/x loads are chunked so compute
   starts as soon as the first chunks land, with a short dummy-matmul
   warm-up keeping the PE clock (HAM) at full rate through the DMA lead-in.
 - q,k are produced as [dh, tok]; scores are computed transposed
   (S^T = k^T q) so softmax's k-reduction lands on matmul-friendly axes.
 - softmax: exp on the scalar engine (scale folded in); the denominator is
   a pairwise fp16 tree-sum of the exp tiles on DVE plus one all-ones
   matmul partition-reduce; normalize via fast approximate reciprocal.
 - The PE is kept at full duty through phase B by interleaving one extra
   matmul per kt slot: o-proj matmuls of the previous token stripe (and,
   for the first stripe, the deferred stripe-3 q-projection chains).
"""

import sys

if "/opt/trn_rl_repo" not in sys.path:
    sys.path.insert(0, "/opt/trn_rl_repo")

import numpy as np

import concourse.bass as bass
import concourse.mybir as mybir
import concourse.tile as tile
from concourse.tile import add_dep_helper
from concourse import bacc

B, N, HID, H = 2, 2048, 2048, 16
DH = 128
HPC = 4  # heads per core
P = 128
F16 = mybir.dt.float16
F32 = mybir.dt.float32
SCALE = 1.0 / float(np.sqrt(DH))

_NC_CACHE = [None]


def build_nc():
    nc = bacc.Bacc(None, target_bir_lowering=False)

    xT = nc.dram_tensor("xT", [HID, N], F16, kind="ExternalInput")
    wqkT = nc.dram_tensor("wqkT", [HID, 2 * HPC * DH], F16, kind="ExternalInput")
    wvT = nc.dram_tensor("wvT", [HID, HPC * DH], F16, kind="ExternalInput")
    woT = nc.dram_tensor("woT", [HPC * DH, HID], F16, kind="ExternalInput")
    cosT = nc.dram_tensor("cosT", [DH, N], F32, kind="ExternalInput")
    sinT = nc.dram_tensor("sinT", [DH, N], F32, kind="ExternalInput")
    outT = nc.dram_tensor("outT", [HID, N], F32, kind="ExternalOutput")

    KT = HID // P  # 16 contraction tiles over hid
    NT = N // P  # 16 token tiles
    NS = N // 512  # 4 token stripes
    MQK = 2 * HPC  # 8 output dim-tiles for q+k

    mult = mybir.AluOpType.mult
    add = mybir.AluOpType.add
    Exp = mybir.ActivationFunctionType.Exp

    xT3 = xT[:].rearrange("(kt p) n -> kt p n", p=P)
    wvT3 = wvT[:].rearrange("(kt p) m -> kt p m", p=P)

    with tile.TileContext(nc) as tc:
        with (
            tc.tile_pool(name="const", bufs=1) as const,
            tc.tile_pool(name="persist", bufs=1) as persist,
        ):
            ones_sb = const.tile([P, P], F16)
            nc.vector.memset(ones_sb[:], 1.0)
            warm_src = const.tile([P, 512], F16, tag="warmsrc")
            nc.vector.memset(warm_src[:], 0.0)

            # -------- startup loads, chunked --------
            x0_tiles = []
            for kt in range(KT):
                xk = persist.tile([P, 512], F16, tag=f"x0_{kt}", name=f"x0_{kt}")
                nc.sync.dma_start(xk[:], xT3[kt, :, 0:512])
                x0_tiles.append(xk)
            # wqk split per 128-wide output block: the m=0 chain can start
            # after just 0.5MB instead of the whole 4MB.
            wqkm_tiles = []

            def load_wqkm(m):
                wk = const.tile(
                    [P, KT, P], F16, tag=f"wqkm{m}", name=f"wqkm{m}"
                )
                nc.sync.dma_start(
                    wk[:],
                    wqkT[:, m * P : (m + 1) * P].rearrange(
                        "(kt p) c -> p kt c", p=P
                    ),
                )
                wqkm_tiles.append(wk)

            cos_tiles, sin_tiles = [], []

            def load_tables(t):
                ck = const.tile([P, 512], F32, tag=f"cos{t}", name=f"cos{t}")
                nc.sync.dma_start(ck[:], cosT[:, t * 512 : (t + 1) * 512])
                cos_tiles.append(ck)
                sk = const.tile([P, 512], F32, tag=f"sin{t}", name=f"sin{t}")
                nc.sync.dma_start(sk[:], sinT[:, t * 512 : (t + 1) * 512])
                sin_tiles.append(sk)

            load_wqkm(0)
            load_wqkm(1)
            load_tables(0)  # RoPE of the first chain needs stripe-0 tables
            for m in range(2, MQK):
                load_wqkm(m)

            # persistent intermediates
            qk_tiles = [
                persist.tile([P, N], F16, tag=f"qk{m}", name=f"qk{m}")
                for m in range(MQK)
            ]
            v_sb = persist.tile([P, NT, HPC * DH], F16, tag="v")

            # ---------------- Phase A: QKV + RoPE ----------------
            # (stripe 3's q chains are deferred into phase B's filler slots)
            with (
                tc.tile_pool(name="aphase", bufs=2) as aphase,
                tc.tile_pool(name="psumA", bufs=3, space="PSUM") as psumA,
            ):
                wv_tiles = []
                for kt in range(KT):
                    wk = aphase.tile(
                        [P, HPC * DH], F16, tag=f"wv{kt}", name=f"wv{kt}", bufs=1
                    )
                    nc.sync.dma_start(wk[:], wvT3[kt])
                    wv_tiles.append(wk)
                for t in range(1, NS):
                    load_tables(t)

                warm_ps = psumA.tile([P, 512], F32, tag="warm", bufs=1)
                for _ in range(48):
                    nc.tensor.matmul(
                        warm_ps[:], ones_sb[:], warm_src[:], start=True, stop=True
                    )

                for s in range(NS):
                    if s == 0:
                        x_tiles = x0_tiles
                    else:
                        xs = aphase.tile(
                            [P, KT, 512], F16, tag="xs", name="xs", bufs=2
                        )
                        nc.sync.dma_start(
                            xs[:],
                            xT[:, s * 512 : (s + 1) * 512].rearrange(
                                "(kt p) n -> p kt n", p=P
                            ),
                        )
                        x_tiles = [xs[:, kt, :] for kt in range(KT)]
                    sl = slice(s * 512, (s + 1) * 512)
                    m_list = range(MQK) if s < NS - 1 else range(HPC, MQK)
                    for m in m_list:
                        ps = psumA.tile([P, 512], F32, tag="ps")
                        for kt in range(KT):
                            nc.tensor.matmul(
                                ps[:],
                                wqkm_tiles[m][:, kt, :],
                                x_tiles[kt][:],
                                start=(kt == 0),
                                stop=(kt == KT - 1),
                            )
                        # RoPE: out = ps*cos + rot(ps)*sin_signed
                        rot = aphase.tile([P, 512], F32, tag="rot")
                        nc.scalar.copy(rot[0:64, :], ps[64:128, :])
                        nc.scalar.copy(rot[64:128, :], ps[0:64, :])
                        t1 = aphase.tile([P, 512], F32, tag="t1")
                        nc.vector.tensor_tensor(t1[:], rot[:], sin_tiles[s][:], mult)
                        t2 = aphase.tile([P, 512], F32, tag="t2")
                        nc.vector.tensor_tensor(t2[:], ps[:], cos_tiles[s][:], mult)
                        nc.vector.tensor_tensor(
                            qk_tiles[m][:, sl], t1[:], t2[:], add
                        )
                    for tt in range(4):
                        tok = s * 4 + tt
                        psv = psumA.tile([P, 512], F32, tag="psv", bufs=2)
                        for kt in range(KT):
                            nc.tensor.matmul(
                                psv[:],
                                x_tiles[kt][:, tt * P : (tt + 1) * P],
                                wv_tiles[kt][:],
                                start=(kt == 0),
                                stop=(kt == KT - 1),
                            )
                        nc.scalar.copy(v_sb[:, tok, :], psv[:])

            # ---------------- Phases B (attention) + C (o-proj) ----------
            with tc.tile_pool(name="late", bufs=3) as late:
                # x stripe-3 reload (chunked) for the deferred q chains
                x3r_tiles = []
                for kt in range(KT):
                    xk = late.tile(
                        [P, 512], F16, tag=f"x3r{kt}", name=f"x3r{kt}", bufs=1
                    )
                    nc.sync.dma_start(
                        xk[:], xT3[kt, :, (NS - 1) * 512 : NS * 512]
                    )
                    x3r_tiles.append(xk)
                wo_sb = late.tile([P, HPC, HID], F16, tag="wo", bufs=1)
                nc.sync.dma_start(
                    wo_sb[:], woT[:].rearrange("(ht p) o -> p ht o", p=P)
                )
                attn_tiles = [
                    late.tile([P, N], F16, tag=f"attn{h}", name=f"attn{h}", bufs=1)
                    for h in range(HPC)
                ]

                with tc.tile_pool(name="psumB", bufs=2, space="PSUM") as psumB:
                    pending = []  # deferred denominator/normalize closure args

                    def emit_denorm(accs, outp, h, qsl):
                        stride = 1
                        while stride < 8:
                            for i in range(0, 8, 2 * stride):
                                nc.vector.tensor_tensor(
                                    accs[i][:], accs[i][:], accs[i + stride][:], add
                                )
                            stride *= 2
                        den = psumB.tile([P, 512], F32, tag="den", bufs=1, name="den")
                        nc.tensor.matmul(
                            den[:], ones_sb[:], accs[0][:], start=True, stop=True
                        )
                        rec = late.tile([P, 512], F32, tag="rec", bufs=2)
                        nc.vector.reciprocal_approx_fast(rec[:], den[:])
                        nc.vector.tensor_tensor(
                            attn_tiles[h][:, qsl], outp[:], rec[:], mult
                        )

                    def make_c_fillers(ts, tag="cps"):
                        # one closure per o-proj matmul for token stripe ts,
                        # spread through the next stripe's kt slots to keep
                        # the PE at full duty.
                        tsl = slice(ts * 512, (ts + 1) * 512)
                        state = {}
                        fillers = []

                        def mk(ho, hi, tag="cps"):
                            def emit():
                                if hi == 0:
                                    state[ho] = psumB.tile(
                                        [P, 512], F32, tag=tag, bufs=2, name="cps"
                                    )
                                nc.tensor.matmul(
                                    state[ho][:],
                                    wo_sb[:, hi, ho * P : (ho + 1) * P],
                                    attn_tiles[hi][:, tsl],
                                    start=(hi == 0),
                                    stop=(hi == HPC - 1),
                                )
                                if hi == HPC - 1:
                                    ob = late.tile([P, 512], F32, tag="co")
                                    nc.vector.tensor_copy(ob[:], state[ho][:])
                                    nc.sync.dma_start(
                                        outT[ho * P : (ho + 1) * P, tsl], ob[:]
                                    )
                            return emit

                        for ho in range(HID // P):
                            for hi in range(HPC):
                                fillers.append(mk(ho, hi, tag=tag))
                        return fillers

                    def make_q3_fillers():
                        # deferred stripe-3 q-projection chains (m=0..3),
                        # one matmul per filler slot + RoPE at chain end.
                        s3 = NS - 1
                        sl3 = slice(s3 * 512, (s3 + 1) * 512)
                        state = {}
                        fillers = []

                        def mk(m, kt):
                            def emit():
                                if kt == 0:
                                    state[m] = psumB.tile(
                                        [P, 512], F32, tag="cps", bufs=2, name="q3ps"
                                    )
                                nc.tensor.matmul(
                                    state[m][:],
                                    wqkm_tiles[m][:, kt, :],
                                    x3r_tiles[kt][:],
                                    start=(kt == 0),
                                    stop=(kt == KT - 1),
                                )
                                if kt == KT - 1:
                                    ps = state[m]
                                    rot = late.tile([P, 512], F32, tag="rot3")
                                    nc.vector.tensor_copy(rot[0:64, :], ps[64:128, :])
                                    nc.vector.tensor_copy(rot[64:128, :], ps[0:64, :])
                                    t1 = late.tile([P, 512], F32, tag="t13")
                                    nc.vector.tensor_tensor(
                                        t1[:], rot[:], sin_tiles[s3][:], mult
                                    )
                                    t2 = late.tile([P, 512], F32, tag="t23")
                                    nc.vector.tensor_tensor(
                                        t2[:], ps[:], cos_tiles[s3][:], mult
                                    )
                                    nc.vector.tensor_tensor(
                                        qk_tiles[m][:, sl3], t1[:], t2[:], add
                                    )
                            return emit

                        for m in range(HPC):
                            for kt in range(KT):
                                fillers.append(mk(m, kt))
                        return fillers

                    for qs in range(NS):
                        qsl = slice(qs * 512, (qs + 1) * 512)
                        fillers = (
                            make_c_fillers(qs - 1) if qs > 0 else make_q3_fillers()
                        )
                        fi = 0
                        for h in range(HPC):
                            kT_tile = qk_tiles[HPC + h]
                            qT_tile = qk_tiles[h]
                            outp = psumB.tile([P, 512], F32, tag="po", bufs=2)
                            accs = []
                            prev_pt = None
                            prev_pv = None
                            for kt in range(NT):
                                st = psumB.tile([P, 512], F32, tag="st", bufs=3)
                                nc.tensor.matmul(
                                    st[:],
                                    kT_tile[:, kt * P : (kt + 1) * P],
                                    qT_tile[:, qsl],
                                    start=True,
                                    stop=True,
                                )
                                pt = late.tile(
                                    [P, 512], F16,
                                    tag=("ptA" if kt % 2 == 0 else "ptB"),
                                    name="pt", bufs=3,
                                )
                                nc.scalar.activation(pt[:], st[:], Exp, scale=SCALE)
                                pv = nc.tensor.matmul(
                                    outp[:],
                                    v_sb[:, kt, h * DH : (h + 1) * DH],
                                    pt[:],
                                    start=(kt == 0),
                                    stop=(kt == NT - 1),
                                )
                                if kt % 2 == 0:
                                    prev_pt = pt
                                    prev_pv = pv
                                else:
                                    acc = late.tile(
                                        [P, 512], F16, tag="acc", bufs=10, name="acc"
                                    )
                                    ai = nc.vector.tensor_tensor(
                                        acc[:], prev_pt[:], pt[:], add
                                    )
                                    add_dep_helper(ai.ins, prev_pv.ins,
                                                   reason="pt SBUF contention")
                                    add_dep_helper(ai.ins, pv.ins,
                                                   reason="pt SBUF contention")
                                    accs.append(acc)
                                if kt == 3 and pending:
                                    emit_denorm(*pending.pop())
                                if (h > 0 or kt >= 6) and fi < len(fillers):
                                    fillers[fi]()
                                    fi += 1
                            pending.append((accs, outp, h, qsl))
                        while fi < len(fillers):
                            fillers[fi]()
                            fi += 1
                    emit_denorm(*pending.pop())
                    # final stripe's o-proj: alternate over the now-free psum
                    # tags for deeper pipelining of the tail burst
                    tail_f = []
                    for tag in ("cps", "po"):
                        tail_f.append(make_c_fillers(NS - 1, tag=tag))
                    for g in range(HID // P):
                        for hi in range(HPC):
                            tail_f[g % 2][g * HPC + hi]()

    nc.finalize()
    return nc


def get_nc():
    if _NC_CACHE[0] is None:
        _NC_CACHE[0] = build_nc()
    return _NC_CACHE[0]


def make_in_maps(hidden_states, cos, sin, w_qkv, w_o):
    """Build the 8 per-core input maps (host-side shard + transpose + cast)."""
    hidden_states = np.asarray(hidden_states, dtype=np.float32)
    cos = np.asarray(cos, dtype=np.float32)
    sin = np.asarray(sin, dtype=np.float32)
    w_qkv = np.asarray(w_qkv, dtype=np.float32)
    w_o = np.asarray(w_o, dtype=np.float32)

    cosT = np.ascontiguousarray(cos.T)  # [DH, N]
    sinT_signed = np.ascontiguousarray(
        np.concatenate([-sin.T[: DH // 2], sin.T[DH // 2 :]], axis=0)
    )

    xT = [
        np.ascontiguousarray(hidden_states[b].T).astype(np.float16)
        for b in range(B)
    ]

    in_maps = []
    for c in range(8):
        b, g = divmod(c, 4)
        qrows = slice(g * HPC * DH, (g + 1) * HPC * DH)
        krows = slice(HID + g * HPC * DH, HID + (g + 1) * HPC * DH)
        vrows = slice(2 * HID + g * HPC * DH, 2 * HID + (g + 1) * HPC * DH)
        wqkT = (
            np.concatenate([w_qkv[qrows], w_qkv[krows]], axis=0)
            .T.astype(np.float16)
        )
        wvT = w_qkv[vrows].T.astype(np.float16)
        woT = w_o[:, g * HPC * DH : (g + 1) * HPC * DH].T.astype(np.float16)
        in_maps.append(
            {
                "xT": xT[b],
                "wqkT": np.ascontiguousarray(wqkT),
                "wvT": np.ascontiguousarray(wvT),
                "woT": np.ascontiguousarray(woT),
                "cosT": cosT,
                "sinT": sinT_signed,
            }
        )
    return in_maps


def assemble_output(results):
    """Sum the 4 o-proj partials per batch and transpose back."""
    out = np.zeros((B, N, HID), dtype=np.float32)
    for c, res in enumerate(results):
        b = c // 4
        out[b] += res["outT"].T
    return out


def kernel(hidden_states, cos, sin, w_qkv, w_o):
    from concourse.bass_utils import run_bass_kernel_spmd

    nc = get_nc()
    in_maps = make_in_maps(hidden_states, cos, sin, w_qkv, w_o)
    res = run_bass_kernel_spmd(nc, in_maps, core_ids=list(range(8)))
    return assemble_output(res.results)
